# revision 40
# baseline (speedup 1.0000x reference)
"""AttnBlock (GroupNorm + 8-head self-attention + proj + residual) on 8 trn2 cores.

Sharding: one attention head per core, both batch elements on every core.
Each core computes its head's contribution to the output projection
(o_head @ Wo[:, head].T) as a full-shape UNNORMALIZED partial plus the
per-(batch, query) softmax denominators; the host divides each partial by
its denominators, sums the 8 results, adds bo and the residual x.

Per-core layouts (partition dim first):
  qT/kT    [128, 4096] bf16   rows 0:64 = batch0 head, rows 64:128 = batch1
  x8       [128, 4, 4096] fp8e4 per batch (gpsimd cast-DMA copy of x): the
           q/k projections run as fp8 DoubleRow matmuls (K=256, 4x fewer PE
           cycles than bf16 K=128); V keeps the bf16 path for accuracy.
  v2       [128 j, 16, 2, 80] fp8e4 per batch; col 64 = 1.0 (softmax denom);
           V bias folded into the pv psum accumulation as a K=1 ones-matmul
           so the v2 write is a pure cast.
  S.T      psum [128 j, 512] f32 per (batch, jt), 2 tags x ring-2
  P pairs  [128, 2, 512] fp8e5 per batch: exp(S*1/8 - 2.5), softmax-shift-
           invariant. Engine per (batch, jt) is a balance knob (KNOBS):
           ScalarE true Exp or the DVE bits trick (e5m2 bits =
           trunc(S*A5 + B5) as uint8; e5m2's 32-octave range needs no
           clamping for |s_scaled| < 7.9). Both produce the same value, so
           assignment is free and tuned per phase to equalize ACT/DVE.
  o.T      psum [65, 512] f32 per batch: rows 0:64 unnormalized o.T,
           row 64 = softmax denominator. All PV matmuls are fp8 DoubleRow
           (K=256), halving PE time vs bf16.
  tail     NO on-device normalization: rows 0:65 cast to bf16 in one op,
           denominator row DMA'd out (dnm), division on the host. bf16 is
           scale-free so the unnormalized magnitudes cost no precision.

GroupNorm is never applied to x: the per-channel affine h = scl*x + bia is
folded into the QKV weight panels (scale W columns by scl, project bia into
a per-output bias). Stats run during the x DMA window, split ScalarE/DVE
for latency; weight-panel DMAs are deferred behind the x tiles.

Steady-state engine balance (TimelineSim): ACT ~200us, DVE ~196us, PE
~175us per rep; chunks 1..7 run at 96-98% ACT/DVE occupancy.
"""

import numpy as np

NUM_HEADS = 8
B, C, H, W = 2, 512, 64, 64
N = H * W            # 4096
HD = C // NUM_HEADS  # 64
GROUPS = 32
EPS = 1e-5
NIC = 8              # i-chunks of 512
NJT = 32             # j-tiles of 128
NJP = NJT // 2       # j-tile PAIRS of 256
CT = 4               # channel tiles of 128
SM_SCALE = 1.0 / 8.0  # 1/sqrt(HD)

# Engine-balance knobs. Each jt's S tile holds BOTH batches [128, 2, 512]
# (2 psum banks, double-buffered); the per-jt exp runs as two 512-wide
# halves on one engine. DVE (bits trick) takes dve_*[phase] of every 16 jt
# (spread via *5 mod 16); ScalarE true-Exp takes the rest. Chunk 0 keeps DVE
# light (it carries GN stats + the k-chunk/v-production stream there).
KNOBS = {
    "b0_dve_c0": 4,       # batch0 exp jts (of 16) on DVE, chunk 0
    "b0_dve_steady": 0,   # batch0 exp jts (of 16) on DVE, chunks 1..7
    "b1_act_c0": 0,       # batch1 exp jts (of 16) on ScalarE, chunk 0
    "b1_act_steady": 1,   # batch1 exp jts (of 16) on ScalarE, chunks 1..7
    "v2_dve": False,      # v2 quantize cast on DVE (else ScalarE)
    "qk_dve": False,      # qk bias+cast on DVE (else ScalarE)
    "ou_dve": True,       # o/denominator cast on DVE (else ScalarE)
    "ob_act": 2,          # how many of the 4 Wo-cast mts go to ScalarE
    "act_stat_n": 2,      # x-stat tiles (in dma order) on ScalarE 2-pass
    "mix_split": 3,       # MIX tile: ScalarE does chunks [0, mix_split)
}

_CACHE = {}


def _make_split_drain_tc(tile_mod, nc):
    """TileContext whose final drain splits its semaphore waits across
    nop instructions (this walrus build rejects >2 waits on one Drain)."""
    from concourse.tile import ScopedClock
    from concourse.tile_sem_assignment import VectorClock

    class SplitDrainTC(tile_mod.TileContext):
        def _drain_and_barrier(self, tick_clock, wait_clock):
            vec = list(
                eval(repr(tick_clock.global_clock).replace("VectorClock(", "").rstrip(")"))
            )
            for i, v in enumerate(vec):
                if v > 0:
                    partial = [v if j == i else 0 for j in range(len(vec))]
                    nop = self.nc.sync.nop()
                    wait_clock.add_sem_waits(
                        nop.ins, ScopedClock({None: VectorClock(partial)})
                    )
            self.nc.sync.drain()
            self.nc.all_engine_barrier()
            popped = self.nc._tile_sem_poison_stack.pop()
            assert popped is self._sem_poison
            self.nc.clear_and_free_semaphores(list(self.sems.allocated().values()))
            self.nc.all_engine_barrier()

    return SplitDrainTC(nc)


def _split_excess_waits(nc, mybir, limit=1):
    """This walrus build rejects >1 sync wait on one instruction; hoist the
    excess onto single-wait NoOps inserted just before, on the same engine."""
    fn = nc.m.functions[0]
    ctr = 0
    for bb in fn.blocks:
        new_insts = []
        changed = False
        for inst in bb.instructions:
            si = inst.sync_info
            if si is not None and si.on_wait and len(si.on_wait) > limit:
                waits = list(si.on_wait)
                excess, keep = waits[:-limit], waits[-limit:]
                for w in excess:
                    nop = mybir.InstNoOp(
                        name=f"waitsplit_{ctr}",
                        engine=inst.engine,
                        sync_info=mybir.SyncInfo(on_wait=[w], on_update=[]),
                    )
                    ctr += 1
                    new_insts.append(nop)
                inst.sync_info = mybir.SyncInfo(
                    on_wait=keep, on_update=list(si.on_update)
                )
                changed = True
            new_insts.append(inst)
        if changed:
            try:
                bb.instructions[:] = new_insts
            except TypeError:
                bb.instructions = new_insts


def build_program(split_waits=True, loops=1):
    import concourse.bass as bass
    import concourse.tile as tile
    from concourse import mybir

    f32 = mybir.dt.float32
    bf16 = mybir.dt.bfloat16
    u8 = mybir.dt.uint8
    fp8e4 = mybir.dt.float8e4
    fp8e5 = mybir.dt.float8e5
    mult = mybir.AluOpType.mult
    add = mybir.AluOpType.add
    subtract = mybir.AluOpType.subtract
    AF = mybir.ActivationFunctionType
    DR = mybir.MatmulPerfMode.DoubleRow

    LOG2E = 1.4426950408889634
    EXP_SHIFT = 2.5
    # e5m2 bits-trick constants (input is RAW S, i.e. pre-1/8 scale):
    # bits = trunc(S*A5 + B5); value = 2^((bits - 60.325 + 0.175 - 0.5)/4)
    A5 = SM_SCALE * LOG2E * 4.0
    B5 = 15 * 4 + 0.5 - 0.175 - EXP_SHIFT * LOG2E * 4.0
    # Wo output-cast engine split: these mt indices go to ScalarE
    ACT_CAST = frozenset(range(KNOBS["ob_act"]))

    nc = bass.Bass("TRN2", debug=False, num_devices=NUM_HEADS)

    xbf = nc.declare_dram_parameter("xbf", [B, C, N], bf16, isOutput=False)
    wq_t = nc.declare_dram_parameter("wq_t", [C, HD], bf16, isOutput=False)
    wk_t = nc.declare_dram_parameter("wk_t", [C, HD], bf16, isOutput=False)
    wv_t = nc.declare_dram_parameter("wv_t", [C, HD], bf16, isOutput=False)
    wo_t = nc.declare_dram_parameter("wo_t", [HD, C], bf16, isOutput=False)
    bqk2 = nc.declare_dram_parameter("bqk2", [128, 2], f32, isOutput=False)  # col0 bq dup, col1 bk dup
    bv_p = nc.declare_dram_parameter("bv", [HD], f32, isOutput=False)
    gam = nc.declare_dram_parameter("gam", [C, 1], f32, isOutput=False)
    bet = nc.declare_dram_parameter("bet", [C, 1], f32, isOutput=False)
    ind16 = nc.declare_dram_parameter("ind16", [128, 8], f32, isOutput=False)
    ind64k = nc.declare_dram_parameter("ind64k", [128, 8], f32, isOutput=False)
    ind32 = nc.declare_dram_parameter("ind32", [128, 8], f32, isOutput=False)
    exp8 = nc.declare_dram_parameter("exp8", [8, 128], f32, isOutput=False)
    out = nc.declare_dram_parameter("out", [B, C, N], bf16, isOutput=True)
    # softmax denominators, normalized on the host: out_full = out / dnm
    dnm = nc.declare_dram_parameter("dnm", [B, N], bf16, isOutput=True)

    tc = _make_split_drain_tc(tile, nc)
    with tc:
        from contextlib import ExitStack

        with ExitStack() as ctx:
            consts = ctx.enter_context(tc.tile_pool(name="consts", bufs=1))
            xpool = ctx.enter_context(tc.tile_pool(name="xpool", bufs=8))
            gnsb = ctx.enter_context(tc.tile_pool(name="gnsb", bufs=4))
            small = ctx.enter_context(tc.tile_pool(name="small", bufs=4))
            ptpool = ctx.enter_context(tc.tile_pool(name="ptpool", bufs=3))
            onpool = ctx.enter_context(tc.tile_pool(name="onpool", bufs=4))
            outp = ctx.enter_context(tc.tile_pool(name="outp", bufs=3))

            # ---------- constants ----------
            # weight-panel DMAs are deferred until after the first rep's x
            # DMAs are queued: x gates the GN stats (critical path), while
            # the panels are only needed ~40us in for the scl folding.
            wq_sb = consts.tile([128, CT, HD], bf16)
            wk_sb = consts.tile([128, CT, HD], bf16)
            wv_sb = consts.tile([128, CT, HD], bf16)
            wo_sb = consts.tile([HD, C], bf16)

            def emit_weight_dmas():
                for wsb, wdr in ((wq_sb, wq_t), (wk_sb, wk_t), (wv_sb, wv_t)):
                    # DRAM [C, HD] row-major -> sbuf [p=128, kt=4, d=64]; c = kt*128+p
                    src = bass.AP(tensor=wdr, offset=0,
                                  ap=[[HD, 128], [128 * HD, CT], [1, HD]])
                    nc.sync.dma_start(out=wsb[:], in_=src)
                nc.sync.dma_start(out=wo_sb[:], in_=wo_t[:, :])

            bqk_sb = consts.tile([128, 2], f32)
            nc.sync.dma_start(out=bqk_sb[:], in_=bqk2[:, :])
            bv_row = consts.tile([1, HD], f32)
            nc.sync.dma_start(
                out=bv_row[:],
                in_=bass.AP(tensor=bv_p, offset=0, ap=[[0, 1], [1, HD]]),
            )
            g_sb = consts.tile([128, CT], f32)
            b_sb = consts.tile([128, CT], f32)
            nc.sync.dma_start(out=g_sb[:], in_=bass.AP(tensor=gam, offset=0, ap=[[1, 128], [128, CT]]))
            nc.sync.dma_start(out=b_sb[:], in_=bass.AP(tensor=bet, offset=0, ap=[[1, 128], [128, CT]]))
            ind16_sb = consts.tile([128, 8], f32)
            nc.sync.dma_start(out=ind16_sb[:], in_=ind16[:, :])
            ind64k_sb = consts.tile([128, 8], f32)
            nc.sync.dma_start(out=ind64k_sb[:], in_=ind64k[:, :])
            ind32_sb = consts.tile([128, 8], f32)
            nc.sync.dma_start(out=ind32_sb[:], in_=ind32[:, :])
            exp8_sb = consts.tile([8, 128], f32)
            nc.sync.dma_start(out=exp8_sb[:], in_=exp8[:, :])
            ones128_sb = consts.tile([1, 128], f32)
            nc.vector.memset(ones128_sb[:], 1.0)
            ones128_bf = consts.tile([1, 128], bf16)
            nc.vector.memset(ones128_bf[:], 1.0)
            eps_sb = consts.tile([8, 1], f32)
            nc.vector.memset(eps_sb[:], EPS)
            negshift_sb = consts.tile([128, 1], f32)
            nc.vector.memset(negshift_sb[:], -EXP_SHIFT)

            actpool = ctx.enter_context(tc.tile_pool(name="actpool", bufs=1))
            for rep in range(loops):
              # persistent activation tensors
              qT = actpool.tile([128, N], bf16, tag="qT", name=f"qT{rep}")
              kT = actpool.tile([128, N], bf16, tag="kT", name=f"kT{rep}")
              v2 = [actpool.tile([128, NJT // 2, 2, 80], fp8e4, tag=f"v2_{b}", name=f"v2_{b}_{rep}") for b in range(B)]
              for b in range(B):
                  nc.gpsimd.memset(v2[b][:], 0.0)
                  nc.gpsimd.memset(v2[b][:, :, :, HD:HD + 1], 1.0)

              x_tiles = [[None] * CT for _ in range(B)]
              x8 = [actpool.tile([128, CT, N], fp8e4, tag=f"x8_{b}", name=f"x8_{b}_{rep}")
                    for b in range(B)]

              # ---------- GroupNorm ----------
              with tc.tile_pool(name="gnps", bufs=2, space="PSUM") as gnps, \
                   tc.tile_pool(name="gnps2", bufs=2, space="PSUM") as gnps2:
                  for b in range(B):
                      for ct in range(CT):
                          x_tiles[b][ct] = xpool.tile([128, N], bf16, tag="xt",
                                                      name=f"xt{b}_{ct}")
                  # tile-major DMA, alternating DVE-stat and ACT-stat tiles so
                  # both engines stream stats from the first megabyte.
                  # Whole-tile transfers: 8KB-contiguous rows per descriptor,
                  # HBM-bandwidth-bound with half the issue overhead of the
                  # old 2048-col chunks.
                  dma_order = [(0, 0), (1, 1), (0, 1), (1, 2), (0, 2), (1, 3), (0, 3), (1, 0)]
                  for b, ct in dma_order:
                      for s in range(2):
                          nc.sync.dma_start(
                              out=x_tiles[b][ct][:, s * 2048:(s + 1) * 2048],
                              in_=xbf[b, ct * 128:(ct + 1) * 128, s * 2048:(s + 1) * 2048])
                  if rep == 0:
                      emit_weight_dmas()
                  # fp8e4 copy of x for the DoubleRow q/k projections:
                  # gpsimd cast-DMA (SWDGE, parallel to the SP HWDGE queue);
                  # channel c = kt*128 + p matches the weight-panel layout
                  for b in range(B):
                      for ct in range(CT):
                          nc.gpsimd.dma_start(out=x8[b][:, ct, :],
                                              in_=xbf[b, ct * 128:(ct + 1) * 128, :])
                  # Stats split across engines: DVE bn_stats for 6 tiles,
                  # ScalarE accumulate-sums for 2. Both paths land in
                  # exs = [m0, m1, m2] such that var_g = avg16(m1) + avg16(m2)
                  # - avg16(m0)^2, mean_g = avg16(m0):
                  #   DVE:  [mean, var, mean^2], indicator 1/16
                  #   ACT:  [sum, sumsq, 0],     indicator 1/(16*4096)
                  # Per-tile engine split, with (0,1) MIXED (ACT does its
                  # first 4 chunks, DVE the rest) to balance latency so the
                  # attention phase starts ASAP after the last x DMA.
                  ACT_STAT = set(dma_order[:KNOBS["act_stat_n"]])
                  MIX_STAT = dma_order[KNOBS["act_stat_n"]]
                  psum_gs = [None] * B
                  for b in range(B):
                      psum_gs[b] = gnps.tile([8, 3 * CT], f32, tag="psg", name=f"psg{b}")

                  def act_stats(b, ct, s_lo, s_hi, ind, start, stop):
                      n = s_hi - s_lo
                      scratch = gnsb.tile([128, 512], bf16, tag="scratch", bufs=2,
                                          name=f"scr{b}_{ct}")
                      acc8 = gnsb.tile([128, 2, 8], f32, tag=f"acc8_{b}{ct}",
                                       name=f"acc8_{b}{ct}")
                      for s in range(s_lo, s_hi):
                          nc.scalar.activation(out=scratch[:], in_=x_tiles[b][ct][:, s * 512:(s + 1) * 512],
                                               func=AF.Identity, accum_out=acc8[:, 0, s - s_lo:s - s_lo + 1])
                          nc.scalar.activation(out=scratch[:], in_=x_tiles[b][ct][:, s * 512:(s + 1) * 512],
                                               func=AF.Square, accum_out=acc8[:, 1, s - s_lo:s - s_lo + 1])
                      exs = gnsb.tile([128, 3], f32, tag=f"exsa{b}_{ct}", name=f"exsa{b}_{ct}")
                      nc.vector.reduce_sum(out=exs[:, 0:2], in_=acc8[:, :, 0:n],
                                           axis=mybir.AxisListType.X)
                      nc.gpsimd.memset(exs[:, 2:3], 0.0)
                      nc.tensor.matmul(psum_gs[b][:, 3 * ct:3 * ct + 3], ind[:], exs[:],
                                       start=start, stop=stop)

                  def dve_stats(b, ct, s_lo, s_hi, ind, start, stop):
                      n = s_hi - s_lo
                      stats = gnsb.tile([128, 8, 6], f32, tag=f"stats{b}_{ct}",
                                        name=f"stats{b}_{ct}")
                      for s in range(s_lo, s_hi):
                          nc.vector.bn_stats(out=stats[:, s - s_lo, :],
                                             in_=x_tiles[b][ct][:, s * 512:(s + 1) * 512])
                      mv = gnsb.tile([128, 2], f32, tag="mv")
                      nc.vector.bn_aggr(out=mv[:], in_=stats[:, 0:n, :])
                      exs = gnsb.tile([128, 3], f32, tag=f"exsd{b}_{ct}", name=f"exsd{b}_{ct}")
                      nc.vector.tensor_copy(out=exs[:, 0:2], in_=mv[:])
                      nc.vector.tensor_tensor(out=exs[:, 2:3], in0=mv[:, 0:1],
                                              in1=mv[:, 0:1], op=mult)
                      nc.tensor.matmul(psum_gs[b][:, 3 * ct:3 * ct + 3], ind[:], exs[:],
                                       start=start, stop=stop)

                  ms = KNOBS["mix_split"]
                  if 0 < ms < 8 and ms != 4:
                      # DVE mix indicator: weight (8-ms)/8 of the tile mean
                      ind_mix_sb = consts.tile([128, 8], f32, tag="indmix")
                      nc.vector.tensor_scalar(out=ind_mix_sb[:], in0=ind16_sb[:],
                                              scalar1=(8 - ms) / 8.0, scalar2=None,
                                              op0=mult)
                  else:
                      ind_mix_sb = ind32_sb
                  for b, ct in dma_order:
                      if (b, ct) in ACT_STAT:
                          act_stats(b, ct, 0, 8, ind64k_sb, True, True)
                      elif (b, ct) == MIX_STAT and 0 < ms < 8:
                          act_stats(b, ct, 0, ms, ind64k_sb, True, False)
                          dve_stats(b, ct, ms, 8, ind_mix_sb, False, True)
                      else:
                          dve_stats(b, ct, 0, 8, ind16_sb, True, True)
                  # Batched scl/bia chain: process all 4 channel-tiles of a
                  # batch in [8, CT]-wide ops (one Ln + one Exp + a few TTs)
                  # instead of ~12 serialized tiny ops per (b, ct).
                  scl_t = [[None] * CT for _ in range(B)]
                  bia_t = [[None] * CT for _ in range(B)]
                  for b in range(B):
                      gst2 = gnsb.tile([8, 3, CT], f32, tag="gst", name=f"gst{b}")
                      for j in range(3):
                          nc.vector.tensor_copy(out=gst2[:, j, :],
                                                in_=psum_gs[b][:, j:3 * CT:3])
                      varg = small.tile([8, CT], f32, tag="varg", name=f"varg{b}")
                      sq0 = small.tile([8, CT], f32, tag="sq0")
                      nc.vector.tensor_tensor(out=varg[:], in0=gst2[:, 1, :], in1=gst2[:, 2, :], op=add)
                      nc.vector.tensor_tensor(out=sq0[:], in0=gst2[:, 0, :], in1=gst2[:, 0, :], op=mult)
                      nc.vector.tensor_tensor(out=varg[:], in0=varg[:], in1=sq0[:], op=subtract)
                      # rstd = exp(-0.5 * ln(var + eps)); Ln+Exp share one ACT table set
                      lnv = small.tile([8, CT], f32, tag="lnv")
                      nc.scalar.activation(out=lnv[:], in_=varg[:], func=AF.Ln, bias=eps_sb[:])
                      gv = small.tile([8, 2, CT], f32, tag="gv")
                      nc.scalar.activation(out=gv[:, 1, :], in_=lnv[:], func=AF.Exp, scale=-0.5)
                      nc.vector.tensor_copy(out=gv[:, 0, :], in_=gst2[:, 0, :])
                      psum_e = gnps2.tile([128, 2, CT], f32, tag="pse", bufs=1, name=f"pse{b}")
                      nc.tensor.matmul(psum_e[:], exp8_sb[:], gv[:], start=True, stop=True)
                      scl_all = small.tile([128, CT], f32, tag=f"scl{b}", name=f"scl{b}")
                      tmp = small.tile([128, CT], f32, tag="tmp")
                      bia_bf = small.tile([128, CT], bf16, tag=f"biabf{b}", name=f"biabf{b}")
                      nc.vector.tensor_tensor(out=scl_all[:], in0=psum_e[:, 1, :], in1=g_sb[:], op=mult)
                      nc.vector.tensor_tensor(out=tmp[:], in0=psum_e[:, 0, :], in1=scl_all[:], op=mult)
                      nc.vector.tensor_tensor(out=bia_bf[:], in0=b_sb[:], in1=tmp[:], op=subtract)
                      for ct in range(CT):
                          scl_t[b][ct] = scl_all[:, ct:ct + 1]
                          bia_t[b][ct] = bia_bf[:, ct:ct + 1]

                  # GroupNorm's per-channel affine h = scl*x + bia folds into the
                  # QKV projections: scale the weight panels by scl along C, and
                  # add the projected bias (W @ bia + b) as a per-output bias.
                  wq_s = [consts.tile([128, CT, HD], fp8e4, tag=f"wqs{b}", name=f"wqs{b}") for b in range(B)]
                  wk_s = [consts.tile([128, CT, HD], fp8e4, tag=f"wks{b}", name=f"wks{b}") for b in range(B)]
                  wv8_s = [consts.tile([128, CT, HD], fp8e4, tag=f"wvs{b}", name=f"wvs{b}") for b in range(B)]
                  for b in range(B):
                      for ws, wsb in ((wq_s, wq_sb), (wk_s, wk_sb), (wv8_s, wv_sb)):
                          for ct in range(CT):
                              nc.vector.tensor_scalar(out=ws[b][:, ct, :], in0=wsb[:, ct, :],
                                                      scalar1=scl_t[b][ct], scalar2=None,
                                                      op0=mult)
                  # q/k bias vectors: [128, 2] = (W @ bia per batch-half) + b
                  bvec_ps = gnps2.tile([128, 2], f32, tag="bvec", bufs=1)
                  for col, wsb in ((0, wq_sb), (1, wk_sb)):
                      for b in range(B):
                          for ct in range(CT):
                              nc.tensor.matmul(bvec_ps[b * 64:(b + 1) * 64, col:col + 1],
                                               wsb[:, ct, :], bia_t[b][ct],
                                               start=(ct == 0), stop=(ct == CT - 1),
                                               tile_position=(0, 64 * b),
                                               skip_group_check=(b == 1))
                  qk_bias = consts.tile([128, 2], f32, tag="qkbias")
                  nc.vector.tensor_tensor(out=qk_bias[:], in0=bvec_ps[:], in1=bqk_sb[:], op=add)
                  # v bias row [1, HD] bf16 per b: folded into the pv psum
                  # accumulation as a K=1 ones-matmul in emit_v_jt, so the
                  # v2 quantize becomes a pure cast (engine-free choice).
                  vb_rows = [None] * B
                  for b in range(B):
                      vb_ps = gnps2.tile([1, HD], f32, tag="vbtmp", bufs=1, name=f"vbps{b}")
                      for ct in range(CT):
                          nc.tensor.matmul(vb_ps[:], bia_t[b][ct], wv_sb[:, ct, :],
                                           start=(ct == 0), stop=(ct == CT - 1))
                      vb_rows[b] = consts.tile([1, HD], bf16, tag=f"vbrow{b}", name=f"vbrow{b}")
                      nc.vector.tensor_tensor(out=vb_rows[b][:], in0=vb_ps[:], in1=bv_row[:], op=add)

              # ---------- QKV + attention + output projection ----------
              # PSUM budget (8 banks): per-batch st tiles [128, 512] x 2
              # tags, double-buffered (4) + o_ps (2) + wops ring (2).
              with tc.tile_pool(name="stps", bufs=2, space="PSUM") as stps, \
                   tc.tile_pool(name="ops", bufs=2, space="PSUM") as ops, \
                   tc.tile_pool(name="wops", bufs=2, space="PSUM") as wops:
                  def emit_v_jt(b, jt):
                      # fp8 DoubleRow over channel pairs: 2 K=256 matmuls
                      # instead of 4 K=128 bf16 ones
                      pv = wops.tile([128, HD], f32, tag="w", name=f"pv{b}_{jt}")
                      for kt in (0, 2):
                          nc.tensor.matmul(pv[:],
                                           x8[b][:, kt:kt + 2, jt * 128:(jt + 1) * 128],
                                           wv8_s[b][:, kt:kt + 2, :],
                                           start=(kt == 0), stop=False,
                                           perf_mode=DR)
                      # bias via K=1 ones-matmul; v2 write is then a pure cast
                      # on ScalarE (idle during chunk 0's v-production phase)
                      nc.tensor.matmul(pv[:], ones128_bf[:], vb_rows[b][:],
                                       start=False, stop=True)
                      if KNOBS["v2_dve"]:
                          nc.vector.tensor_copy(out=v2[b][:, jt // 2, jt % 2, 0:HD],
                                                in_=pv[:])
                      else:
                          nc.scalar.activation(out=v2[b][:, jt // 2, jt % 2, 0:HD],
                                               in_=pv[:], func=AF.Identity)

                  def emit_qk_chunk(which, ic):
                      # fp8 DoubleRow: 2 K=256 matmuls per batch instead of
                      # 4 K=128 bf16 ones -- 4x fewer PE cycles. DoubleRow
                      # dst must start at partition 0 (s3d3 ISA check), so
                      # each batch gets its own [64, 512] psum tile; the bias
                      # casts split per batch across both engines.
                      ws, dest, bcol = ((wq_s, qT, 0), (wk_s, kT, 1))[which]
                      pqs = []
                      for b in range(B):
                          pq = wops.tile([64, 512], f32, tag="w",
                                         name=f"pq{which}_{ic}_{b}")
                          for kt in (0, 2):
                              nc.tensor.matmul(pq[:], ws[b][:, kt:kt + 2, :],
                                               x8[b][:, kt:kt + 2, ic * 512:(ic + 1) * 512],
                                               start=(kt == 0), stop=(kt == 2),
                                               tile_position=(0, 0),
                                               skip_group_check=(b == 1),
                                               perf_mode=DR)
                          pqs.append(pq)
                      for b in range(B):
                          dst = dest[b * 64:(b + 1) * 64, ic * 512:(ic + 1) * 512]
                          bias = qk_bias[b * 64:(b + 1) * 64, bcol:bcol + 1]
                          if b == (1 if KNOBS["qk_dve"] else 0):
                              nc.vector.tensor_scalar(out=dst, in0=pqs[b][:],
                                                      scalar1=bias,
                                                      scalar2=None, op0=add)
                          else:
                              nc.scalar.activation(out=dst, in_=pqs[b][:],
                                                   func=AF.Identity, bias=bias)

                  def emit_exp_half(eng_dve, dst, src):
                      if eng_dve:
                          nc.vector.tensor_scalar(out=dst.bitcast(u8), in0=src,
                                                  scalar1=A5, scalar2=B5,
                                                  op0=mult, op1=add)
                      else:
                          nc.scalar.activation(out=dst, in_=src,
                                               func=AF.Exp, scale=SM_SCALE,
                                               bias=negshift_sb[:])

                  def emit_st_jt(ic, jt, pc_box):
                      # Per-batch st tiles on separate ring-2 tags so the two
                      # batches' S matmul + exp streams run concurrently on
                      # both engines; the exp engine per (batch, jt) is a
                      # balance knob (ACT true-Exp and the DVE e5m2 bits
                      # trick produce the same exp(S/8 - 2.5) fp8e5).
                      if jt % 2 == 0:
                          pc_box[0] = [ptpool.tile([128, 2, 512], fp8e5, tag=f"pt{b}",
                                                   name=f"pt_{ic}_{jt // 2}_{b}")
                                       for b in range(B)]
                      p2 = pc_box[0]
                      spread = (jt * 5) % 16
                      b0_dve = spread < (KNOBS["b0_dve_c0"] if ic == 0 else KNOBS["b0_dve_steady"])
                      b1_act = spread < (KNOBS["b1_act_c0"] if ic == 0 else KNOBS["b1_act_steady"])
                      st0 = stps.tile([128, 512], f32, tag="st0", name=f"st{ic}_{jt}_0")
                      nc.tensor.matmul(st0[:],
                                       kT[0:64, jt * 128:(jt + 1) * 128],
                                       qT[0:64, ic * 512:(ic + 1) * 512],
                                       start=True, stop=True, tile_position=(0, 0))
                      emit_exp_half(b0_dve, p2[0][:, jt % 2, :], st0[:])
                      st1 = stps.tile([128, 512], f32, tag="st1", name=f"st{ic}_{jt}_1")
                      nc.tensor.matmul(st1[:],
                                       kT[64:128, jt * 128:(jt + 1) * 128],
                                       qT[64:128, ic * 512:(ic + 1) * 512],
                                       start=True, stop=True, tile_position=(64, 0))
                      emit_exp_half(not b1_act, p2[1][:, jt % 2, :], st1[:])
                      return p2 if jt % 2 == 1 else None

                  def emit_pv(o_ps, jt, p2):
                      if p2 is None:
                          return
                      jp = jt // 2
                      first = (jp == 0)
                      last = (jp == NJP - 1)
                      for b in range(B):
                          nc.tensor.matmul(o_ps[b][:],
                                           v2[b][:, jp, :, 0:HD + 1],
                                           p2[b][:, :, :],
                                           start=first, stop=last,
                                           perf_mode=DR)

                  def emit_tail_head(ic, o_ps):
                      # No on-device softmax normalization: the unnormalized
                      # o.T (rows 0:64) plus the denominator row (64) are cast
                      # psum->sbuf bf16 in ONE 512-wide op; the denominator
                      # row is DMA'd out and the HOST divides the final
                      # per-core partial by it (a per-(b,i) scalar commutes
                      # with the Wo projection; bf16 is scale-free so the
                      # unnormalized magnitudes cost no precision).
                      oN = [None] * B
                      for b in range(B):
                          with nc.allow_low_precision(reason="unnormalized o + denom to bf16; scale-free"):
                              oN[b] = onpool.tile([HD + 1, 512], bf16, tag="oN",
                                                  name=f"oN{ic}_{b}")
                              if KNOBS["ou_dve"]:
                                  nc.vector.tensor_copy(out=oN[b][:], in_=o_ps[b][:])
                              else:
                                  nc.scalar.activation(out=oN[b][:], in_=o_ps[b][:],
                                                       func=AF.Identity)
                          nc.sync.dma_start(out=dnm[b, ic * 512:(ic + 1) * 512],
                                            in_=oN[b][HD:HD + 1, :])
                      return oN

                  def emit_tail_wo(ic, oN, b, mt, cast_act=None):
                      # One Wo projection matmul + psum->sbuf cast + DMA out.
                      # Emitted interspersed through the next chunk's jt loop
                      # as always-ready PE filler (HAM stays un-throttled).
                      wp = wops.tile([128, 512], f32, tag="w", name=f"wp{ic}_{b}_{mt}")
                      nc.tensor.matmul(wp[:], wo_sb[:, mt * 128:(mt + 1) * 128],
                                       oN[b][0:HD, :],
                                       start=True, stop=True)
                      ob = outp.tile([128, 512], bf16, tag="ob", name=f"ob{ic}_{b}_{mt}")
                      if (mt in ACT_CAST) if cast_act is None else cast_act:
                          nc.scalar.activation(out=ob[:], in_=wp[:], func=AF.Identity)
                      else:
                          nc.vector.tensor_copy(out=ob[:], in_=wp[:])
                      nc.sync.dma_start(
                          out=out[b, mt * 128:(mt + 1) * 128, ic * 512:(ic + 1) * 512],
                          in_=ob[:])

                  # chunk 0 interleaves the k-chunk / v-tile production with its
                  # own S/exp/PV stream so attention starts as soon as q0+k0+v[0]
                  # exist, instead of after the whole QKV phase. q(ic+1) is
                  # produced DURING chunk ic (not all up front) so later chunks
                  # keep some exp-independent PE work in flight.
                  emit_qk_chunk(0, 0)
                  o_ps0 = [ops.tile([HD + 1, 512], f32, tag="ops", name=f"ops0_{bb}") for bb in range(B)]
                  pc_box = [None]
                  for jt in range(NJT):
                      if jt % 4 == 0:
                          emit_qk_chunk(1, jt // 4)
                      emit_v_jt(0, jt)
                      emit_v_jt(1, jt)
                      pc = emit_st_jt(0, jt, pc_box)
                      emit_pv(o_ps0, jt, pc)
                      if jt == 21:
                          emit_qk_chunk(0, 1)

                  PRE = 4  # S/exp tiles emitted before the previous chunk's tail
                  prev = (0, o_ps0)
                  for ic in range(1, NIC):
                      pc_box = [None]
                      pts = [emit_st_jt(ic, jt, pc_box) for jt in range(PRE)]
                      oN_prev = emit_tail_head(prev[0], prev[1])
                      o_ps = [ops.tile([HD + 1, 512], f32, tag="ops", name=f"ops{ic}_{bb}") for bb in range(B)]
                      for jt in range(PRE):
                          emit_pv(o_ps, jt, pts[jt])
                      if ic in (1, 3, 5):
                          # TWO chunks' q-production as one 16-matmul dep-free
                          # burst at the chunk boundary (where the tail chain
                          # makes PE work thin): >= 6.8us of gapless PE work
                          # covers a full free-running HAM activity window, so
                          # a cold-throttled PE deterministically re-warms.
                          emit_qk_chunk(0, ic + 1)
                          emit_qk_chunk(0, ic + 2)
                      wo_q = [(bb, mt) for bb in range(B) for mt in range(CT)]
                      for jt in range(PRE, NJT):
                          pc = emit_st_jt(ic, jt, pc_box)
                          emit_pv(o_ps, jt, pc)
                          if jt % 2 == 1 and jt >= 5 and wo_q:
                              bb, mt = wo_q.pop(0)
                              emit_tail_wo(prev[0], oN_prev, bb, mt)
                      prev = (ic, o_ps)
                  # final tail is fully exposed (nothing left to overlap):
                  # alternate the casts across both engines to halve its span
                  oN_last = emit_tail_head(prev[0], prev[1])
                  for i, (bb, mt) in enumerate([(bb, mt) for bb in range(B)
                                                for mt in range(CT)]):
                      emit_tail_wo(prev[0], oN_last, bb, mt, cast_act=(i % 2 == 0))
    if split_waits:
        _split_excess_waits(nc, mybir)
    return nc


def _prep_in_maps(inputs):
    from concourse import mybir

    np_bf16 = mybir.dt.np(mybir.dt.bfloat16)
    x = np.asarray(inputs["x"], np.float32)
    gamma = np.asarray(inputs["gamma"], np.float32)
    beta = np.asarray(inputs["beta"], np.float32)
    Wq = np.asarray(inputs["Wq"], np.float32)
    bq = np.asarray(inputs["bq"], np.float32)
    Wk = np.asarray(inputs["Wk"], np.float32)
    bk = np.asarray(inputs["bk"], np.float32)
    Wv = np.asarray(inputs["Wv"], np.float32)
    bv = np.asarray(inputs["bv"], np.float32)
    Wo = np.asarray(inputs["Wo"], np.float32)

    xbf = np.ascontiguousarray(x.reshape(B, C, N)).astype(np_bf16)
    ind16 = np.zeros((128, 8), np.float32)
    for p in range(128):
        ind16[p, p // 16] = 1.0 / 16.0
    ind64k = ind16 / 4096.0
    ind32 = ind16 / 2.0
    exp8 = np.zeros((8, 128), np.float32)
    for p in range(128):
        exp8[p // 16, p] = 1.0
    gam2 = np.ascontiguousarray(gamma.reshape(C, 1))
    bet2 = np.ascontiguousarray(beta.reshape(C, 1))

    in_maps = []
    for c in range(NUM_HEADS):
        sl = slice(c * HD, (c + 1) * HD)
        bqk2 = np.stack([np.tile(bq[sl], 2), np.tile(bk[sl], 2)], axis=1)
        in_maps.append({
            "xbf": xbf,
            "wq_t": np.ascontiguousarray(Wq[sl, :].T).astype(np_bf16),
            "wk_t": np.ascontiguousarray(Wk[sl, :].T).astype(np_bf16),
            "wv_t": np.ascontiguousarray(Wv[sl, :].T).astype(np_bf16),
            "wo_t": np.ascontiguousarray(Wo[:, sl].T).astype(np_bf16),
            "bqk2": np.ascontiguousarray(bqk2, dtype=np.float32),
            "bv": np.ascontiguousarray(bv[sl]),
            "gam": gam2,
            "bet": bet2,
            "ind16": ind16,
            "ind64k": ind64k,
            "ind32": ind32,
            "exp8": exp8,
        })
    return in_maps


def kernel(**inputs):
    from concourse.bass_utils import run_bass_kernel_spmd

    if "nc" not in _CACHE:
        _CACHE["nc"] = build_program()
    nc = _CACHE["nc"]
    in_maps = _prep_in_maps(inputs)
    res = run_bass_kernel_spmd(nc, in_maps, core_ids=list(range(NUM_HEADS)))
    x = np.asarray(inputs["x"], np.float32)
    bo = np.asarray(inputs["bo"], np.float32)
    acc = np.zeros((B, C, N), np.float32)
    for c in range(NUM_HEADS):
        # per-head softmax denominators: normalize the unnormalized partial
        rc = 1.0 / res.results[c]["dnm"].astype(np.float32)  # [B, N]
        acc += res.results[c]["out"].astype(np.float32) * rc[:, None, :]
    acc += bo[None, :, None]
    return (x + acc.reshape(B, C, H, W)).astype(np.float32)

